# revision 24
# baseline (speedup 1.0000x reference)
"""Int8 per-token-quantized linear (MluQuantLinearInt8) on 8 Trainium2 cores.

  out[s, n] = (sum_k q[s,k] * w[n,k]) * x_scale[s] * w_scale[n]
  q = round(x / x_scale) clipped to [-127, 127],  x_scale = max(|x|_row, 1e-8)/127

Sharding: data-parallel over tokens (512/core); weights replicated, streamed
once per core. Weights are host-pretransposed to [K, N] bf16 (int8 values are
exact in bf16, so int8xint8 products accumulate exactly in fp32 PSUM).

Schedule: x tiles stream first, split across BOTH hardware DGE queues
(sync + scalar engines, ~300 GB/s each); the per-tile quant chain is split
into K-halves across vector/scalar; q is transposed on the PE (idle in the
head) into PSUM bf16 staging tiles instead of the XBAR, whose issue blocks
an engine for the whole transfer and contends with the bulk-load queues;
the x_scale row broadcast is built on-chip (PE transpose + ones-matmul);
weight-chunk 0's GEMM runs at 128-column granularity per token tile as each
tile's qT lands (full matmul cadence measured at this width); chunks 1..31
run 512-column steady state at the PE roofline. tile_wait_until sim-time
floors keep the Tile scheduler's greedy CoreSim pass (whose DMA model is
optimistic) from ordering later tiles' work ahead of earlier tiles' on the
in-order engines.
"""

import sys
from contextlib import ExitStack
from functools import lru_cache

import numpy as np

for _p in ("/opt/trn_rl_repo", "/root/.axon_site/_ro/trn_rl_repo"):
    if _p not in sys.path:
        sys.path.append(_p)

import ml_dtypes  # noqa: E402

import concourse.bass as bass  # noqa: E402
import concourse.bass2jax as bass2jax  # noqa: E402
import concourse.mybir as mybir  # noqa: E402
import concourse.tile as tile  # noqa: E402
from concourse.bass_utils import (  # noqa: E402
    compile_bir_kernel as _orig_compile_bir_kernel,
    run_bass_kernel_spmd,
)
from concourse.masks import make_identity  # noqa: E402

# The walrus build in this container accepts only ONE sync-wait per
# instruction ("Too many sync wait commands", CoreV3GenImpl setupSyncWait) —
# Tile's kernel-tail drain carries several. Split extra waits onto preceding
# single-wait EventSemaphore carriers on the same engine (engine program order
# makes the AND of waits equivalent).
import json as _json  # noqa: E402


def _split_multi_waits(bir_json):
    d = _json.loads(bir_json)
    changed = False
    for fn in d.get("functions", []):
        for bb in fn.get("blocks", []) or []:
            insts = bb.get("instructions")
            if not insts:
                continue
            out = []
            for ins in insts:
                si = ins.get("sync_info")
                waits = (si or {}).get("on_wait") or []
                if len(waits) > 1:
                    for j, w in enumerate(waits[:-1]):
                        out.append(
                            {
                                "engine": ins.get("engine"),
                                "ins": [],
                                "outs": [],
                                "name": f"{ins.get('name', 'I')}_w{j}",
                                "opcode": "EventSemaphore",
                                "sync_info": {"on_update": [], "on_wait": [w]},
                            }
                        )
                    si["on_wait"] = [waits[-1]]
                    changed = True
                out.append(ins)
            bb["instructions"] = out
    if not changed:
        return bir_json
    return _json.dumps(d).encode()


def _patched_compile_bir_kernel(bir_json, tmpdir, neff_name="file.neff"):
    return _orig_compile_bir_kernel(
        _split_multi_waits(bir_json), tmpdir, neff_name=neff_name
    )


bass2jax.compile_bir_kernel = _patched_compile_bir_kernel

P = 128
NCORES = 8
S, K_FULL, N_FULL = 4096, 4096, 16384
QMAX = 127.0
MAGIC = 12582912.0  # 1.5 * 2**23: (y + MAGIC) - MAGIC == RNE-round(y) for |y| < 2**22
F32 = mybir.dt.float32
BF16 = mybir.dt.bfloat16


def build_nc(S_C, K, N, NSUB=4, exact_divide=True, early_start=True):
    """One-core program; SPMD-replicated across cores by the runner.

    Inputs (per core):
      x   [S_C, K]  f32 - this core's token slice
      wt  [WC, P, KC, NSUB*P] bf16 - weights, host-packed as SBUF-layout chunks
      ws  [P, NT]   f32 - weight_scale packed ws[p, nt] = weight_scale[nt*128+p]
    Output:
      outT [N, S_C] f32 - dequantized output, transposed
    """
    KC = K // P  # contraction chunks
    TT = S_C // P  # token tiles
    NT = N // P  # output-channel tiles (one psum tile each)
    WC = NT // NSUB  # streamed weight chunks
    KH = K // 2  # half the contraction dim (elements)
    KCH = KC // 2  # half the contraction dim (128-chunks)

    nc = bass.Bass()
    x = nc.declare_dram_parameter("x", [S_C, K], F32, isOutput=False)
    wt = nc.declare_dram_parameter("wt", [WC, P, KC, NSUB * P], BF16, isOutput=False)
    ws = nc.declare_dram_parameter("ws", [P, NT], F32, isOutput=False)
    outT = nc.declare_dram_parameter("outT", [N, S_C], F32, isOutput=True)

    outT_t = outT.rearrange("(nt p) s -> nt p s", p=P)

    with tile.TileContext(nc) as tc, ExitStack() as ctx:
        const_pool = ctx.enter_context(tc.tile_pool(name="const", bufs=1))
        xpool = ctx.enter_context(tc.tile_pool(name="xp", bufs=4))
        qpool = ctx.enter_context(tc.tile_pool(name="qp", bufs=2))
        qt_pool = ctx.enter_context(tc.tile_pool(name="qt", bufs=1))
        wpool = ctx.enter_context(tc.tile_pool(name="wp", bufs=2))
        opool = ctx.enter_context(tc.tile_pool(name="op", bufs=4))
        spool = ctx.enter_context(tc.tile_pool(name="sp", bufs=1))
        ps_pool = ctx.enter_context(tc.tile_pool(name="psp", bufs=5, space="PSUM"))
        xs_psp = ctx.enter_context(tc.tile_pool(name="xsps", bufs=1, space="PSUM"))
        tpsp = ctx.enter_context(tc.tile_pool(name="tpsp", bufs=1, space="PSUM"))

        # ---- DMA issue order: x tiles first (they pace everything); w0
        # early (chunk 0 runs per-tile during the head); w1 after the x
        # stream. Each load is split across the sync and scalar queues.
        def load_wchunk(wc):
            wtile = wpool.tile([P, KC, NSUB * P], BF16, name="wtile")
            nc.sync.dma_start(wtile[:, :KCH], wt[wc, :, :KCH])
            nc.scalar.dma_start(wtile[:, KCH:], wt[wc, :, KCH:])
            return wtile

        # NOTE: DMA issues are deliberately NOT floored — a floor on a load
        # lets unfloored loads jump ahead of it in the hardware queue order.
        xts = []
        for t in range(TT):
            xt = xpool.tile([P, K], F32, name="xt")
            KQ = K // 4
            rows = x[t * P : (t + 1) * P]
            nc.sync.dma_start(xt[:, :KQ], rows[:, :KQ])
            nc.scalar.dma_start(xt[:, KH : KH + KQ], rows[:, KH : KH + KQ])
            nc.sync.dma_start(xt[:, KQ:KH], rows[:, KQ:KH])
            nc.scalar.dma_start(xt[:, KH + KQ :], rows[:, KH + KQ :])
            xts.append(xt)
            if t == 0:
                ws_sb = const_pool.tile([P, NT], F32)
                nc.sync.dma_start(ws_sb, ws[:, :])
                wtiles = {0: load_wchunk(0)}
        wtiles[1] = load_wchunk(1)  # lands right after x3; needed at ~60us

        ident_f32 = const_pool.tile([P, P], F32)
        make_identity(nc, ident_f32)
        ident_bf16 = const_pool.tile([P, P], BF16)
        make_identity(nc, ident_bf16)
        ones_row = const_pool.tile([1, P], F32)
        nc.vector.memset(ones_row, 1.0)
        # preload the scalar engine's Copy activation table so tile 0's
        # quant doesn't pay the ~1.3us ACT_TABLE_LOAD on its critical path
        act_warm = const_pool.tile([1, P], F32)
        nc.scalar.activation(
            act_warm, ones_row, mybir.ActivationFunctionType.Copy,
            bias=0.0, scale=1.0,
        )

        qT = qt_pool.tile([P, TT, KC, P], BF16)
        xs_all = spool.tile([P, TT], F32)  # xs_all[p, t] = x_scale[t*128+p]
        xsb = spool.tile([P, S_C], F32)  # xsb[p, tok] = x_scale[tok]

        inv127 = float(np.float32(1.0 / 127.0))

        def quant_compute(t):
            """amax -> x_scale -> q (bf16). vector: reduces+smalls+sub_a;
            scalar: pre_a, pre_b, sub_b. Returns the q tile."""
            xt = xts[t]
            KQ = K // 4
            amax_a = spool.tile([P, 1], F32, name="amax_a")
            amax_b = spool.tile([P, 1], F32, name="amax_b")
            amax_c = spool.tile([P, 1], F32, name="amax_c")
            amax_d = spool.tile([P, 1], F32, name="amax_d")
            qbounds = [(0, KQ, amax_a), (KH, KH + KQ, amax_c),
                       (KQ, KH, amax_b), (KH + KQ, K, amax_d)]
            for lo, hi, am in qbounds:
                nc.vector.tensor_reduce(
                    out=am,
                    in_=xt[:, lo:hi],
                    axis=mybir.AxisListType.X,
                    op=mybir.AluOpType.max,
                    apply_absolute_value=True,
                )
            nc.vector.tensor_tensor(amax_a, amax_a, amax_c, op=mybir.AluOpType.max)
            nc.vector.tensor_tensor(amax_b, amax_b, amax_d, op=mybir.AluOpType.max)
            amax = spool.tile([P, 1], F32, name="amax")
            nc.vector.tensor_tensor(amax, amax_a, amax_b, op=mybir.AluOpType.max)
            # xs = max(amax, 1e-8)/127 (fused); inv = 1/xs = 127/amax exactly
            # the quantizer scale (~1ulp, matches reference within tolerance)
            nc.vector.tensor_scalar(
                xs_all[:, t : t + 1], amax, 1e-8, inv127,
                op0=mybir.AluOpType.max, op1=mybir.AluOpType.mult,
            )
            inv = spool.tile([P, 1], F32, name="inv")
            nc.vector.reciprocal(inv, xs_all[:, t : t + 1])

            # x_scale broadcast for this tile's 128 tokens, all on-chip:
            # [P,1] column -> PE transpose -> [1,P] row -> ones-matmul -> [P,P]
            xs_ps = xs_psp.tile([P, P], F32, name="xs_ps")
            nc.tensor.transpose(xs_ps[0:1, :], xs_all[:, t : t + 1], ident_f32)
            xs_row = spool.tile([1, P], F32, name="xs_row")
            nc.vector.tensor_copy(xs_row, xs_ps[0:1, :])
            nc.tensor.matmul(xs_ps, lhsT=ones_row, rhs=xs_row, start=True, stop=True)
            nc.vector.tensor_copy(xsb[:, t * P : (t + 1) * P], xs_ps)

            # q = round(x * (127/amax)) via the +MAGIC/-MAGIC RNE trick
            q = qpool.tile([P, K], BF16, name="q")
            nc.scalar.activation(
                xt[:, :KH], xt[:, :KH], mybir.ActivationFunctionType.Copy,
                bias=MAGIC, scale=inv,
            )
            nc.vector.tensor_scalar(
                q[:, :KH], xt[:, :KH], MAGIC, None, op0=mybir.AluOpType.subtract
            )
            nc.scalar.activation(
                xt[:, KH:], xt[:, KH:], mybir.ActivationFunctionType.Copy,
                bias=MAGIC, scale=inv,
            )
            nc.scalar.activation(
                q[:, KH:], xt[:, KH:], mybir.ActivationFunctionType.Copy,
                bias=-MAGIC, scale=1.0,
            )
            return q

        def transpose_half(q, t, half, evict_engine):
            """PE-transpose one K-half of q into qT via a PSUM staging tile.
            The XBAR is avoided entirely: it shares the DMA fabric/queues and
            its issue blocks an engine for the whole transfer."""
            tps = tpsp.tile([P, KCH, P], BF16, name="tps")
            for j in range(KCH):
                kc = half * KCH + j
                nc.tensor.transpose(
                    tps[:, j, :], q[:, kc * P : (kc + 1) * P], ident_bf16
                )
            dst = qT[:, t, half * KCH : (half + 1) * KCH]
            hh = KCH // 2
            if evict_engine == "vector":
                nc.vector.tensor_copy(dst[:, :hh], tps[:, :hh])
                nc.vector.tensor_copy(dst[:, hh:], tps[:, hh:])
            else:
                nc.scalar.copy(dst[:, :hh], tps[:, :hh])
                nc.scalar.copy(dst[:, hh:], tps[:, hh:])

        def dequant_store(ps, nt, c0, c1):
            """out[:, c0:c1] = (psum * w_scale[nt]) * x_scale[tok]; DMA out."""
            out_sb = opool.tile([P, S_C], F32, name="osb")
            nc.vector.scalar_tensor_tensor(
                out=out_sb[:, c0:c1],
                in0=ps,
                scalar=ws_sb[:, nt : nt + 1],
                in1=xsb[:, c0:c1],
                op0=mybir.AluOpType.mult,
                op1=mybir.AluOpType.mult,
            )
            nc.sync.dma_start(outT_t[nt][:, c0:c1], out_sb[:, c0:c1])

        # ---- Head: per-tile quant + transpose, with weight-chunk 0's GEMM
        # interleaved at 128-col granularity (full matmul cadence measured
        # at this width) so the PE works while x tiles stream in.
        w0 = wtiles[0]
        TFLOOR = [0.017, 0.036, 0.044, 0.052]
        for t in range(TT):
            with tc.tile_wait_until(TFLOOR[t]):
                q = quant_compute(t)
                transpose_half(q, t, 0, "vector")
                pss = []
                for sub in range(NSUB):
                    ps = ps_pool.tile([P, S_C], F32, name="ps")
                    pss.append(ps)
                    for kc in range(KCH):
                        nc.tensor.matmul(
                            ps[:, :P],
                            lhsT=w0[:, kc, sub * P : (sub + 1) * P],
                            rhs=qT[:, t, kc, :],
                            start=(kc == 0),
                            stop=False,
                        )
                transpose_half(q, t, 1, "scalar")
                for sub in range(NSUB):
                    ps = pss[sub]
                    for kc in range(KCH, KC):
                        nc.tensor.matmul(
                            ps[:, :P],
                            lhsT=w0[:, kc, sub * P : (sub + 1) * P],
                            rhs=qT[:, t, kc, :],
                            start=False,
                            stop=(kc == KC - 1),
                        )
                    dequant_store(ps[:, :P], sub, t * P, (t + 1) * P)
        wtiles.pop(0)

        # ---- Steady state: streamed weights-stationary GEMM, 512-col ----
        HT = TT // 2
        for wc in range(1, WC):
            wtile = wtiles.pop(wc) if wc in wtiles else load_wchunk(wc)
            for sub in range(NSUB):
                nt = wc * NSUB + sub
                if wc == WC - 1 and sub == NSUB - 1:
                    # last psum group split in column halves so the first
                    # half dequant+store overlaps the second half's matmuls
                    # (trims the kernel tail)
                    psA = ps_pool.tile([P, S_C], F32, name="ps")
                    psB = ps_pool.tile([P, S_C], F32, name="ps")
                    for kc in range(KC):
                        nc.tensor.matmul(
                            psA[:, : S_C // 2],
                            lhsT=wtile[:, kc, sub * P : (sub + 1) * P],
                            rhs=qT[:, :HT, kc, :],
                            start=(kc == 0),
                            stop=(kc == KC - 1),
                        )
                    dequant_store(psA[:, : S_C // 2], nt, 0, S_C // 2)
                    for kc in range(KC):
                        nc.tensor.matmul(
                            psB[:, : S_C // 2],
                            lhsT=wtile[:, kc, sub * P : (sub + 1) * P],
                            rhs=qT[:, HT:, kc, :],
                            start=(kc == 0),
                            stop=(kc == KC - 1),
                        )
                    dequant_store(psB[:, : S_C // 2], nt, S_C // 2, S_C)
                    continue
                ps = ps_pool.tile([P, S_C], F32, name="ps")
                for kc in range(KC):
                    nc.tensor.matmul(
                        ps,
                        lhsT=wtile[:, kc, sub * P : (sub + 1) * P],
                        rhs=qT[:, :, kc, :],
                        start=(kc == 0),
                        stop=(kc == KC - 1),
                    )
                dequant_store(ps, nt, 0, S_C)

    return nc


def pack_inputs(input_tensor, weight, weight_scale, S_C, K, N, NSUB=4):
    """Host-side prep: shard x, pack weights to bf16 SBUF-chunk layout."""
    KC = K // P
    NT = N // P
    WC = NT // NSUB
    x = np.ascontiguousarray(input_tensor.reshape(-1, K))  # [S, K]
    w_bf = weight.astype(ml_dtypes.bfloat16)  # [N, K], int8 values exact
    # pack[wc, p, kc, n] = w[wc*NSUB*P + n, kc*P + p]
    wt = np.ascontiguousarray(
        w_bf.reshape(WC, NSUB * P, KC, P).transpose(0, 3, 2, 1)
    )
    ws = np.ascontiguousarray(
        weight_scale.reshape(NT, P).T.astype(np.float32)
    )  # [P, NT]
    return x, wt, ws


@lru_cache(maxsize=2)
def _compiled_nc(S_C, K, N, NSUB, exact_divide):
    return build_nc(S_C, K, N, NSUB=NSUB, exact_divide=exact_divide)


def run(input_tensor, weight, weight_scale, n_cores=NCORES, trace=False,
        exact_divide=True, NSUB=4):
    Sfull, K = input_tensor.shape[-2], input_tensor.shape[-1]
    N = weight.shape[0]
    S_C = Sfull // n_cores
    x, wt, ws = pack_inputs(input_tensor, weight, weight_scale, S_C, K, N, NSUB)
    nc = _compiled_nc(S_C, K, N, NSUB, exact_divide)
    in_maps = [
        {"x": np.ascontiguousarray(x[c * S_C : (c + 1) * S_C]), "wt": wt, "ws": ws}
        for c in range(n_cores)
    ]
    res = run_bass_kernel_spmd(nc, in_maps, core_ids=list(range(n_cores)), trace=trace)
    out = np.empty((Sfull, N), np.float32)
    for c in range(n_cores):
        out[c * S_C : (c + 1) * S_C] = res.results[c]["outT"].T
    return out[None], res


def kernel(input_tensor, weight, weight_scale):
    out, _ = run(
        np.asarray(input_tensor), np.asarray(weight), np.asarray(weight_scale)
    )
    return out


# revision 26
# speedup vs baseline: 1.0113x; 1.0113x over previous
"""Int8 per-token-quantized linear (MluQuantLinearInt8) on 8 Trainium2 cores.

  out[s, n] = (sum_k q[s,k] * w[n,k]) * x_scale[s] * w_scale[n]
  q = round(x / x_scale) clipped to [-127, 127],  x_scale = max(|x|_row, 1e-8)/127

Sharding: data-parallel over tokens (512/core); weights replicated, streamed
once per core. Weights are host-pretransposed to [K, N] bf16 (int8 values are
exact in bf16, so int8xint8 products accumulate exactly in fp32 PSUM).

Schedule: x tiles stream first, split across BOTH hardware DGE queues
(sync + scalar engines, ~300 GB/s each); the per-tile quant chain is split
into K-halves across vector/scalar; q is transposed on the PE (idle in the
head) into PSUM bf16 staging tiles instead of the XBAR, whose issue blocks
an engine for the whole transfer and contends with the bulk-load queues;
the x_scale row broadcast is built on-chip (PE transpose + ones-matmul);
weight-chunk 0's GEMM runs at 128-column granularity per token tile as each
tile's qT lands (full matmul cadence measured at this width); chunks 1..31
run 512-column steady state at the PE roofline. tile_wait_until sim-time
floors keep the Tile scheduler's greedy CoreSim pass (whose DMA model is
optimistic) from ordering later tiles' work ahead of earlier tiles' on the
in-order engines.
"""

import sys
from contextlib import ExitStack
from functools import lru_cache

import numpy as np

for _p in ("/opt/trn_rl_repo", "/root/.axon_site/_ro/trn_rl_repo"):
    if _p not in sys.path:
        sys.path.append(_p)

import ml_dtypes  # noqa: E402

import concourse.bass as bass  # noqa: E402
import concourse.bass2jax as bass2jax  # noqa: E402
import concourse.mybir as mybir  # noqa: E402
import concourse.tile as tile  # noqa: E402
from concourse.bass_utils import (  # noqa: E402
    compile_bir_kernel as _orig_compile_bir_kernel,
    run_bass_kernel_spmd,
)
from concourse.masks import make_identity  # noqa: E402

# The walrus build in this container accepts only ONE sync-wait per
# instruction ("Too many sync wait commands", CoreV3GenImpl setupSyncWait) —
# Tile's kernel-tail drain carries several. Split extra waits onto preceding
# single-wait EventSemaphore carriers on the same engine (engine program order
# makes the AND of waits equivalent).
import json as _json  # noqa: E402


def _split_multi_waits(bir_json):
    d = _json.loads(bir_json)
    changed = False
    for fn in d.get("functions", []):
        for bb in fn.get("blocks", []) or []:
            insts = bb.get("instructions")
            if not insts:
                continue
            out = []
            for ins in insts:
                si = ins.get("sync_info")
                waits = (si or {}).get("on_wait") or []
                if len(waits) > 1:
                    for j, w in enumerate(waits[:-1]):
                        out.append(
                            {
                                "engine": ins.get("engine"),
                                "ins": [],
                                "outs": [],
                                "name": f"{ins.get('name', 'I')}_w{j}",
                                "opcode": "EventSemaphore",
                                "sync_info": {"on_update": [], "on_wait": [w]},
                            }
                        )
                    si["on_wait"] = [waits[-1]]
                    changed = True
                out.append(ins)
            bb["instructions"] = out
    if not changed:
        return bir_json
    return _json.dumps(d).encode()


def _patched_compile_bir_kernel(bir_json, tmpdir, neff_name="file.neff"):
    return _orig_compile_bir_kernel(
        _split_multi_waits(bir_json), tmpdir, neff_name=neff_name
    )


bass2jax.compile_bir_kernel = _patched_compile_bir_kernel

P = 128
NCORES = 8
S, K_FULL, N_FULL = 4096, 4096, 16384
QMAX = 127.0
MAGIC = 12582912.0  # 1.5 * 2**23: (y + MAGIC) - MAGIC == RNE-round(y) for |y| < 2**22
F32 = mybir.dt.float32
BF16 = mybir.dt.bfloat16


def build_nc(S_C, K, N, NSUB=4, exact_divide=True, early_start=True):
    """One-core program; SPMD-replicated across cores by the runner.

    Inputs (per core):
      x   [S_C, K]  f32 - this core's token slice
      wt  [WC, P, KC, NSUB*P] bf16 - weights, host-packed as SBUF-layout chunks
      ws  [P, NT]   f32 - weight_scale packed ws[p, nt] = weight_scale[nt*128+p]
    Output:
      outT [N, S_C] f32 - dequantized output, transposed
    """
    KC = K // P  # contraction chunks
    TT = S_C // P  # token tiles
    NT = N // P  # output-channel tiles (one psum tile each)
    WC = NT // NSUB  # streamed weight chunks
    KH = K // 2  # half the contraction dim (elements)
    KCH = KC // 2  # half the contraction dim (128-chunks)

    nc = bass.Bass()
    x = nc.declare_dram_parameter("x", [S_C, K], F32, isOutput=False)
    wt = nc.declare_dram_parameter("wt", [WC, P, KC, NSUB * P], BF16, isOutput=False)
    ws = nc.declare_dram_parameter("ws", [P, NT], F32, isOutput=False)
    outT = nc.declare_dram_parameter("outT", [N, S_C], F32, isOutput=True)

    outT_t = outT.rearrange("(nt p) s -> nt p s", p=P)

    with tile.TileContext(nc) as tc, ExitStack() as ctx:
        const_pool = ctx.enter_context(tc.tile_pool(name="const", bufs=1))
        xpool = ctx.enter_context(tc.tile_pool(name="xp", bufs=4))
        qpool = ctx.enter_context(tc.tile_pool(name="qp", bufs=2))
        qt_pool = ctx.enter_context(tc.tile_pool(name="qt", bufs=1))
        wpool = ctx.enter_context(tc.tile_pool(name="wp", bufs=2))
        opool = ctx.enter_context(tc.tile_pool(name="op", bufs=4))
        spool = ctx.enter_context(tc.tile_pool(name="sp", bufs=1))
        ps_pool = ctx.enter_context(tc.tile_pool(name="psp", bufs=5, space="PSUM"))
        xs_psp = ctx.enter_context(tc.tile_pool(name="xsps", bufs=1, space="PSUM"))
        tpsp = ctx.enter_context(tc.tile_pool(name="tpsp", bufs=1, space="PSUM"))

        # ---- DMA issue order: x tiles first (they pace everything); w0
        # early (chunk 0 runs per-tile during the head); w1 after the x
        # stream. Each load is split across the sync and scalar queues.
        def load_wchunk(wc):
            wtile = wpool.tile([P, KC, NSUB * P], BF16, name="wtile")
            nc.sync.dma_start(wtile[:, :KCH], wt[wc, :, :KCH])
            nc.scalar.dma_start(wtile[:, KCH:], wt[wc, :, KCH:])
            return wtile

        # NOTE: DMA issues are deliberately NOT floored — a floor on a load
        # lets unfloored loads jump ahead of it in the hardware queue order.
        xts = []
        for t in range(TT):
            xt = xpool.tile([P, K], F32, name="xt")
            nc.sync.dma_start(xt[:, :KH], x[t * P : (t + 1) * P, :KH])
            nc.scalar.dma_start(xt[:, KH:], x[t * P : (t + 1) * P, KH:])
            xts.append(xt)
            if t == 0:
                ws_sb = const_pool.tile([P, NT], F32)
                nc.sync.dma_start(ws_sb, ws[:, :])
                wtiles = {0: load_wchunk(0)}
        wtiles[1] = load_wchunk(1)  # lands right after x3; needed at ~60us

        ident_f32 = const_pool.tile([P, P], F32)
        make_identity(nc, ident_f32)
        ident_bf16 = const_pool.tile([P, P], BF16)
        make_identity(nc, ident_bf16)
        ones_h = const_pool.tile([1, P], mybir.dt.float16)
        nc.gpsimd.memset(ones_h, 1.0)
        ones_row = const_pool.tile([1, P], F32)
        nc.gpsimd.memset(ones_row, 1.0)  # gpsimd: vector is busy with reduces
        # preload the scalar engine's Copy activation table so tile 0's
        # quant doesn't pay the ~1.3us ACT_TABLE_LOAD on its critical path
        act_warm = const_pool.tile([1, P], F32)
        nc.scalar.activation(
            act_warm, ones_row, mybir.ActivationFunctionType.Copy,
            bias=0.0, scale=1.0,
        )

        qT = qt_pool.tile([P, TT, KC, P], BF16)
        xs_all = spool.tile([P, TT], F32)  # xs_all[p, t] = x_scale[t*128+p]
        xsb = spool.tile([P, S_C], F32)  # xsb[p, tok] = x_scale[tok]

        inv127 = float(np.float32(1.0 / 127.0))

        def quant_compute(t):
            """amax -> x_scale -> q (bf16). vector: reduces+smalls+sub_a;
            scalar: pre_a, pre_b, sub_b. Returns the q tile."""
            xt = xts[t]
            amax_a = spool.tile([P, 1], F32, name="amax_a")
            amax_b = spool.tile([P, 1], F32, name="amax_b")
            nc.vector.tensor_reduce(
                out=amax_a,
                in_=xt[:, :KH],
                axis=mybir.AxisListType.X,
                op=mybir.AluOpType.max,
                apply_absolute_value=True,
            )
            nc.vector.tensor_reduce(
                out=amax_b,
                in_=xt[:, KH:],
                axis=mybir.AxisListType.X,
                op=mybir.AluOpType.max,
                apply_absolute_value=True,
            )
            amax = spool.tile([P, 1], F32, name="amax")
            nc.vector.tensor_tensor(amax, amax_a, amax_b, op=mybir.AluOpType.max)
            # xs = max(amax, 1e-8)/127 (fused); inv = 1/xs = 127/amax exactly
            # the quantizer scale (~1ulp, matches reference within tolerance)
            nc.vector.tensor_scalar(
                xs_all[:, t : t + 1], amax, 1e-8, inv127,
                op0=mybir.AluOpType.max, op1=mybir.AluOpType.mult,
            )
            inv = spool.tile([P, 1], F32, name="inv")
            nc.vector.reciprocal(inv, xs_all[:, t : t + 1])

            # x_scale broadcast for this tile's 128 tokens, all on-chip:
            # [P,1] column -> PE transpose -> [1,P] row -> ones-matmul -> [P,P]
            xs_ps = xs_psp.tile([P, P], F32, name="xs_ps")
            nc.tensor.transpose(xs_ps[0:1, :], xs_all[:, t : t + 1], ident_f32)
            # fp16 broadcast matmul: 1 cycle/row vs 4 for fp32 on the cold PE.
            # xsb at fp16 precision adds <=4.9e-4 relative on the output,
            # within the 2e-3 absmax gate (kernel sits at 1.9e-4 otherwise).
            xs_row = spool.tile([1, P], mybir.dt.float16, name="xs_row")
            nc.vector.tensor_copy(xs_row, xs_ps[0:1, :])
            nc.tensor.matmul(xs_ps, lhsT=ones_h, rhs=xs_row, start=True, stop=True)
            nc.vector.tensor_copy(xsb[:, t * P : (t + 1) * P], xs_ps)

            # q = round(x * (127/amax)) via the +MAGIC/-MAGIC RNE trick
            q = qpool.tile([P, K], BF16, name="q")
            nc.scalar.activation(
                xt[:, :KH], xt[:, :KH], mybir.ActivationFunctionType.Copy,
                bias=MAGIC, scale=inv,
            )
            nc.vector.tensor_scalar(
                q[:, :KH], xt[:, :KH], MAGIC, None, op0=mybir.AluOpType.subtract
            )
            nc.scalar.activation(
                xt[:, KH:], xt[:, KH:], mybir.ActivationFunctionType.Copy,
                bias=MAGIC, scale=inv,
            )
            nc.scalar.activation(
                q[:, KH:], xt[:, KH:], mybir.ActivationFunctionType.Copy,
                bias=-MAGIC, scale=1.0,
            )
            return q

        def transpose_half(q, t, half, evict_engine):
            """PE-transpose one K-half of q into qT via a PSUM staging tile.
            The XBAR is avoided entirely: it shares the DMA fabric/queues and
            its issue blocks an engine for the whole transfer."""
            tps = tpsp.tile([P, KCH, P], BF16, name="tps")
            for j in range(KCH):
                kc = half * KCH + j
                nc.tensor.transpose(
                    tps[:, j, :], q[:, kc * P : (kc + 1) * P], ident_bf16
                )
            dst = qT[:, t, half * KCH : (half + 1) * KCH]
            if evict_engine == "vector":
                nc.vector.tensor_copy(dst, tps)
            else:
                nc.scalar.copy(dst, tps)

        def dequant_store(ps, nt, c0, c1):
            """out[:, c0:c1] = (psum * w_scale[nt]) * x_scale[tok]; DMA out."""
            out_sb = opool.tile([P, S_C], F32, name="osb")
            nc.vector.scalar_tensor_tensor(
                out=out_sb[:, c0:c1],
                in0=ps,
                scalar=ws_sb[:, nt : nt + 1],
                in1=xsb[:, c0:c1],
                op0=mybir.AluOpType.mult,
                op1=mybir.AluOpType.mult,
            )
            nc.sync.dma_start(outT_t[nt][:, c0:c1], out_sb[:, c0:c1])

        # ---- Head: per-tile quant + transpose, with weight-chunk 0's GEMM
        # interleaved at 128-col granularity (full matmul cadence measured
        # at this width) so the PE works while x tiles stream in.
        w0 = wtiles[0]
        TFLOOR = [0.017, 0.036, 0.044, 0.052]
        for t in range(TT):
            with tc.tile_wait_until(TFLOOR[t]):
                q = quant_compute(t)
                transpose_half(q, t, 0, "vector")
                pss = []
                for sub in range(NSUB):
                    ps = ps_pool.tile([P, S_C], F32, name="ps")
                    pss.append(ps)
                    for kc in range(KCH):
                        nc.tensor.matmul(
                            ps[:, :P],
                            lhsT=w0[:, kc, sub * P : (sub + 1) * P],
                            rhs=qT[:, t, kc, :],
                            start=(kc == 0),
                            stop=False,
                        )
                transpose_half(q, t, 1, "scalar")
                for sub in range(NSUB):
                    ps = pss[sub]
                    for kc in range(KCH, KC):
                        nc.tensor.matmul(
                            ps[:, :P],
                            lhsT=w0[:, kc, sub * P : (sub + 1) * P],
                            rhs=qT[:, t, kc, :],
                            start=False,
                            stop=(kc == KC - 1),
                        )
                    dequant_store(ps[:, :P], sub, t * P, (t + 1) * P)
        wtiles.pop(0)

        # ---- Steady state: streamed weights-stationary GEMM, 512-col ----
        HT = TT // 2
        for wc in range(1, WC):
            wtile = wtiles.pop(wc) if wc in wtiles else load_wchunk(wc)
            for sub in range(NSUB):
                nt = wc * NSUB + sub
                if wc == WC - 1 and sub == NSUB - 1:
                    # last psum group split in column halves so the first
                    # half dequant+store overlaps the second half's matmuls
                    # (trims the kernel tail)
                    psA = ps_pool.tile([P, S_C], F32, name="ps")
                    psB = ps_pool.tile([P, S_C], F32, name="ps")
                    for kc in range(KC):
                        nc.tensor.matmul(
                            psA[:, : S_C // 2],
                            lhsT=wtile[:, kc, sub * P : (sub + 1) * P],
                            rhs=qT[:, :HT, kc, :],
                            start=(kc == 0),
                            stop=(kc == KC - 1),
                        )
                    dequant_store(psA[:, : S_C // 2], nt, 0, S_C // 2)
                    for kc in range(KC):
                        nc.tensor.matmul(
                            psB[:, : S_C // 2],
                            lhsT=wtile[:, kc, sub * P : (sub + 1) * P],
                            rhs=qT[:, HT:, kc, :],
                            start=(kc == 0),
                            stop=(kc == KC - 1),
                        )
                    dequant_store(psB[:, : S_C // 2], nt, S_C // 2, S_C)
                    continue
                ps = ps_pool.tile([P, S_C], F32, name="ps")
                for kc in range(KC):
                    nc.tensor.matmul(
                        ps,
                        lhsT=wtile[:, kc, sub * P : (sub + 1) * P],
                        rhs=qT[:, :, kc, :],
                        start=(kc == 0),
                        stop=(kc == KC - 1),
                    )
                dequant_store(ps, nt, 0, S_C)

    return nc


def pack_inputs(input_tensor, weight, weight_scale, S_C, K, N, NSUB=4):
    """Host-side prep: shard x, pack weights to bf16 SBUF-chunk layout."""
    KC = K // P
    NT = N // P
    WC = NT // NSUB
    x = np.ascontiguousarray(input_tensor.reshape(-1, K))  # [S, K]
    w_bf = weight.astype(ml_dtypes.bfloat16)  # [N, K], int8 values exact
    # pack[wc, p, kc, n] = w[wc*NSUB*P + n, kc*P + p]
    wt = np.ascontiguousarray(
        w_bf.reshape(WC, NSUB * P, KC, P).transpose(0, 3, 2, 1)
    )
    ws = np.ascontiguousarray(
        weight_scale.reshape(NT, P).T.astype(np.float32)
    )  # [P, NT]
    return x, wt, ws


@lru_cache(maxsize=2)
def _compiled_nc(S_C, K, N, NSUB, exact_divide):
    return build_nc(S_C, K, N, NSUB=NSUB, exact_divide=exact_divide)


def run(input_tensor, weight, weight_scale, n_cores=NCORES, trace=False,
        exact_divide=True, NSUB=4):
    Sfull, K = input_tensor.shape[-2], input_tensor.shape[-1]
    N = weight.shape[0]
    S_C = Sfull // n_cores
    x, wt, ws = pack_inputs(input_tensor, weight, weight_scale, S_C, K, N, NSUB)
    nc = _compiled_nc(S_C, K, N, NSUB, exact_divide)
    in_maps = [
        {"x": np.ascontiguousarray(x[c * S_C : (c + 1) * S_C]), "wt": wt, "ws": ws}
        for c in range(n_cores)
    ]
    res = run_bass_kernel_spmd(nc, in_maps, core_ids=list(range(n_cores)), trace=trace)
    out = np.empty((Sfull, N), np.float32)
    for c in range(n_cores):
        out[c * S_C : (c + 1) * S_C] = res.results[c]["outT"].T
    return out[None], res


def kernel(input_tensor, weight, weight_scale):
    out, _ = run(
        np.asarray(input_tensor), np.asarray(weight), np.asarray(weight_scale)
    )
    return out


# revision 28
# speedup vs baseline: 1.0150x; 1.0037x over previous
"""Int8 per-token-quantized linear (MluQuantLinearInt8) on 8 Trainium2 cores.

  out[s, n] = (sum_k q[s,k] * w[n,k]) * x_scale[s] * w_scale[n]
  q = round(x / x_scale) clipped to [-127, 127],  x_scale = max(|x|_row, 1e-8)/127

Sharding: data-parallel over tokens (512/core); weights replicated, streamed
once per core. Weights are host-pretransposed to [K, N] bf16 (int8 values are
exact in bf16, so int8xint8 products accumulate exactly in fp32 PSUM).

Schedule: x tiles stream first, split across BOTH hardware DGE queues
(sync + scalar engines, ~300 GB/s each); the per-tile quant chain is split
into K-halves across vector/scalar; q is transposed on the PE (idle in the
head) into PSUM bf16 staging tiles instead of the XBAR, whose issue blocks
an engine for the whole transfer and contends with the bulk-load queues;
the x_scale row broadcast is built on-chip (PE transpose + ones-matmul);
weight-chunk 0's GEMM runs at 128-column granularity per token tile as each
tile's qT lands (full matmul cadence measured at this width); chunks 1..31
run 512-column steady state at the PE roofline. tile_wait_until sim-time
floors keep the Tile scheduler's greedy CoreSim pass (whose DMA model is
optimistic) from ordering later tiles' work ahead of earlier tiles' on the
in-order engines.
"""

import sys
from contextlib import ExitStack
from functools import lru_cache

import numpy as np

for _p in ("/opt/trn_rl_repo", "/root/.axon_site/_ro/trn_rl_repo"):
    if _p not in sys.path:
        sys.path.append(_p)

import ml_dtypes  # noqa: E402

import concourse.bass as bass  # noqa: E402
import concourse.bass2jax as bass2jax  # noqa: E402
import concourse.mybir as mybir  # noqa: E402
import concourse.tile as tile  # noqa: E402
from concourse.bass_utils import (  # noqa: E402
    compile_bir_kernel as _orig_compile_bir_kernel,
    run_bass_kernel_spmd,
)
from concourse.masks import make_identity  # noqa: E402

# The walrus build in this container accepts only ONE sync-wait per
# instruction ("Too many sync wait commands", CoreV3GenImpl setupSyncWait) —
# Tile's kernel-tail drain carries several. Split extra waits onto preceding
# single-wait EventSemaphore carriers on the same engine (engine program order
# makes the AND of waits equivalent).
import json as _json  # noqa: E402


def _split_multi_waits(bir_json):
    d = _json.loads(bir_json)
    changed = False
    for fn in d.get("functions", []):
        for bb in fn.get("blocks", []) or []:
            insts = bb.get("instructions")
            if not insts:
                continue
            out = []
            for ins in insts:
                si = ins.get("sync_info")
                waits = (si or {}).get("on_wait") or []
                if len(waits) > 1:
                    for j, w in enumerate(waits[:-1]):
                        out.append(
                            {
                                "engine": ins.get("engine"),
                                "ins": [],
                                "outs": [],
                                "name": f"{ins.get('name', 'I')}_w{j}",
                                "opcode": "EventSemaphore",
                                "sync_info": {"on_update": [], "on_wait": [w]},
                            }
                        )
                    si["on_wait"] = [waits[-1]]
                    changed = True
                out.append(ins)
            bb["instructions"] = out
    if not changed:
        return bir_json
    return _json.dumps(d).encode()


def _patched_compile_bir_kernel(bir_json, tmpdir, neff_name="file.neff"):
    return _orig_compile_bir_kernel(
        _split_multi_waits(bir_json), tmpdir, neff_name=neff_name
    )


bass2jax.compile_bir_kernel = _patched_compile_bir_kernel

P = 128
NCORES = 8
S, K_FULL, N_FULL = 4096, 4096, 16384
QMAX = 127.0
MAGIC = 12582912.0  # 1.5 * 2**23: (y + MAGIC) - MAGIC == RNE-round(y) for |y| < 2**22
F32 = mybir.dt.float32
BF16 = mybir.dt.bfloat16


def build_nc(S_C, K, N, NSUB=4, exact_divide=True, early_start=True):
    """One-core program; SPMD-replicated across cores by the runner.

    Inputs (per core):
      x   [S_C, K]  f32 - this core's token slice
      wt  [WC, P, KC, NSUB*P] bf16 - weights, host-packed as SBUF-layout chunks
      ws  [P, NT]   f32 - weight_scale packed ws[p, nt] = weight_scale[nt*128+p]
    Output:
      outT [N, S_C] f32 - dequantized output, transposed
    """
    KC = K // P  # contraction chunks
    TT = S_C // P  # token tiles
    NT = N // P  # output-channel tiles (one psum tile each)
    WC = NT // NSUB  # streamed weight chunks
    KH = K // 2  # half the contraction dim (elements)
    KCH = KC // 2  # half the contraction dim (128-chunks)

    nc = bass.Bass()
    x = nc.declare_dram_parameter("x", [S_C, K], F32, isOutput=False)
    wt = nc.declare_dram_parameter("wt", [WC, P, KC, NSUB * P], BF16, isOutput=False)
    ws = nc.declare_dram_parameter("ws", [P, NT], F32, isOutput=False)
    outT = nc.declare_dram_parameter("outT", [N, S_C], F32, isOutput=True)

    outT_t = outT.rearrange("(nt p) s -> nt p s", p=P)

    with tile.TileContext(nc) as tc, ExitStack() as ctx:
        const_pool = ctx.enter_context(tc.tile_pool(name="const", bufs=1))
        xpool = ctx.enter_context(tc.tile_pool(name="xp", bufs=4))
        qpool = ctx.enter_context(tc.tile_pool(name="qp", bufs=2))
        qt_pool = ctx.enter_context(tc.tile_pool(name="qt", bufs=1))
        wpool = ctx.enter_context(tc.tile_pool(name="wp", bufs=2))
        opool = ctx.enter_context(tc.tile_pool(name="op", bufs=4))
        spool = ctx.enter_context(tc.tile_pool(name="sp", bufs=1))
        ps_pool = ctx.enter_context(tc.tile_pool(name="psp", bufs=5, space="PSUM"))
        xs_psp = ctx.enter_context(tc.tile_pool(name="xsps", bufs=1, space="PSUM"))
        tpsp = ctx.enter_context(tc.tile_pool(name="tpsp", bufs=1, space="PSUM"))

        # ---- DMA issue order: x tiles first (they pace everything); w0
        # early (chunk 0 runs per-tile during the head); w1 after the x
        # stream. Each load is split across the sync and scalar queues.
        def load_wchunk(wc):
            wtile = wpool.tile([P, KC, NSUB * P], BF16, name="wtile")
            nc.sync.dma_start(wtile[:, :KCH], wt[wc, :, :KCH])
            nc.scalar.dma_start(wtile[:, KCH:], wt[wc, :, KCH:])
            return wtile

        # NOTE: DMA issues are deliberately NOT floored — a floor on a load
        # lets unfloored loads jump ahead of it in the hardware queue order.
        xts = []
        for t in range(TT):
            xt = xpool.tile([P, K], F32, name="xt")
            nc.sync.dma_start(xt[:, :KH], x[t * P : (t + 1) * P, :KH])
            nc.scalar.dma_start(xt[:, KH:], x[t * P : (t + 1) * P, KH:])
            xts.append(xt)
            if t == 0:
                ws_sb = const_pool.tile([P, NT], F32)
                nc.sync.dma_start(ws_sb, ws[:, :])
                wtiles = {0: load_wchunk(0)}
        wtiles[1] = load_wchunk(1)  # lands right after x3; needed at ~60us

        ident_f32 = const_pool.tile([P, P], F32)
        make_identity(nc, ident_f32)
        ident_bf16 = const_pool.tile([P, P], BF16)
        make_identity(nc, ident_bf16)
        ones_row = const_pool.tile([1, P], F32)
        nc.vector.memset(ones_row, 1.0)
        # preload the scalar engine's Copy activation table so tile 0's
        # quant doesn't pay the ~1.3us ACT_TABLE_LOAD on its critical path
        act_warm = const_pool.tile([1, P], F32)
        nc.scalar.activation(
            act_warm, ones_row, mybir.ActivationFunctionType.Copy,
            bias=0.0, scale=1.0,
        )

        qT = qt_pool.tile([P, TT, KC, P], BF16)
        xs_all = spool.tile([P, TT], F32)  # xs_all[p, t] = x_scale[t*128+p]
        xsb = spool.tile([P, S_C], F32)  # xsb[p, tok] = x_scale[tok]

        inv127 = float(np.float32(1.0 / 127.0))

        def quant_compute(t):
            """amax -> x_scale -> q (bf16). vector: reduces+smalls+sub_a;
            scalar: pre_a, pre_b, sub_b. Returns the q tile."""
            xt = xts[t]
            amax_a = spool.tile([P, 1], F32, name="amax_a")
            amax_b = spool.tile([P, 1], F32, name="amax_b")
            nc.vector.tensor_reduce(
                out=amax_a,
                in_=xt[:, :KH],
                axis=mybir.AxisListType.X,
                op=mybir.AluOpType.max,
                apply_absolute_value=True,
            )
            nc.vector.tensor_reduce(
                out=amax_b,
                in_=xt[:, KH:],
                axis=mybir.AxisListType.X,
                op=mybir.AluOpType.max,
                apply_absolute_value=True,
            )
            amax = spool.tile([P, 1], F32, name="amax")
            nc.vector.tensor_tensor(amax, amax_a, amax_b, op=mybir.AluOpType.max)
            # xs = max(amax, 1e-8)/127 (fused); inv = 1/xs = 127/amax exactly
            # the quantizer scale (~1ulp, matches reference within tolerance)
            nc.vector.tensor_scalar(
                xs_all[:, t : t + 1], amax, 1e-8, inv127,
                op0=mybir.AluOpType.max, op1=mybir.AluOpType.mult,
            )
            inv = spool.tile([P, 1], F32, name="inv")
            nc.vector.reciprocal(inv, xs_all[:, t : t + 1])

            # x_scale broadcast for this tile's 128 tokens, all on-chip:
            # [P,1] column -> PE transpose -> [1,P] row -> ones-matmul -> [P,P]
            xs_ps = xs_psp.tile([P, P], F32, name="xs_ps")
            nc.tensor.transpose(xs_ps[0:1, :], xs_all[:, t : t + 1], ident_f32)
            xs_row = spool.tile([1, P], F32, name="xs_row")
            nc.vector.tensor_copy(xs_row, xs_ps[0:1, :])
            nc.tensor.matmul(xs_ps, lhsT=ones_row, rhs=xs_row, start=True, stop=True)
            nc.vector.tensor_copy(xsb[:, t * P : (t + 1) * P], xs_ps)

            # q = round(x * (127/amax)) via the +MAGIC/-MAGIC RNE trick.
            # Last tile: no further reduces are coming, so the vector engine
            # takes the whole b-half in parallel with scalar's pre_a — this
            # chain gates the 512-col phase start.
            q = qpool.tile([P, K], BF16, name="q")
            last = t == TT - 1
            nc.scalar.activation(
                xt[:, :KH], xt[:, :KH], mybir.ActivationFunctionType.Copy,
                bias=MAGIC, scale=inv,
            )
            nc.vector.tensor_scalar(
                q[:, :KH], xt[:, :KH], MAGIC, None, op0=mybir.AluOpType.subtract
            )
            if last:
                nc.vector.tensor_scalar(
                    xt[:, KH:], xt[:, KH:], inv, MAGIC,
                    op0=mybir.AluOpType.mult, op1=mybir.AluOpType.add,
                )
                nc.vector.tensor_scalar(
                    q[:, KH:], xt[:, KH:], MAGIC, None,
                    op0=mybir.AluOpType.subtract,
                )
            else:
                nc.scalar.activation(
                    xt[:, KH:], xt[:, KH:], mybir.ActivationFunctionType.Copy,
                    bias=MAGIC, scale=inv,
                )
                nc.scalar.activation(
                    q[:, KH:], xt[:, KH:], mybir.ActivationFunctionType.Copy,
                    bias=-MAGIC, scale=1.0,
                )
            return q

        def transpose_half(q, t, half, evict_engine):
            """PE-transpose one K-half of q into qT via a PSUM staging tile.
            The XBAR is avoided entirely: it shares the DMA fabric/queues and
            its issue blocks an engine for the whole transfer."""
            tps = tpsp.tile([P, KCH, P], BF16, name="tps")
            for j in range(KCH):
                kc = half * KCH + j
                nc.tensor.transpose(
                    tps[:, j, :], q[:, kc * P : (kc + 1) * P], ident_bf16
                )
            dst = qT[:, t, half * KCH : (half + 1) * KCH]
            if evict_engine == "vector":
                nc.vector.tensor_copy(dst, tps)
            else:
                nc.scalar.copy(dst, tps)

        def dequant_store(ps, nt, c0, c1):
            """out[:, c0:c1] = (psum * w_scale[nt]) * x_scale[tok]; DMA out."""
            out_sb = opool.tile([P, S_C], F32, name="osb")
            nc.vector.scalar_tensor_tensor(
                out=out_sb[:, c0:c1],
                in0=ps,
                scalar=ws_sb[:, nt : nt + 1],
                in1=xsb[:, c0:c1],
                op0=mybir.AluOpType.mult,
                op1=mybir.AluOpType.mult,
            )
            nc.sync.dma_start(outT_t[nt][:, c0:c1], out_sb[:, c0:c1])

        # ---- Head: per-tile quant + transpose, with weight-chunk 0's GEMM
        # interleaved at 128-col granularity (full matmul cadence measured
        # at this width) so the PE works while x tiles stream in.
        w0 = wtiles[0]
        TFLOOR = [0.017, 0.036, 0.044, 0.052]
        for t in range(TT):
            with tc.tile_wait_until(TFLOOR[t]):
                q = quant_compute(t)
                transpose_half(q, t, 0, "vector")
                pss = []
                for sub in range(NSUB):
                    ps = ps_pool.tile([P, S_C], F32, name="ps")
                    pss.append(ps)
                    for kc in range(KCH):
                        nc.tensor.matmul(
                            ps[:, :P],
                            lhsT=w0[:, kc, sub * P : (sub + 1) * P],
                            rhs=qT[:, t, kc, :],
                            start=(kc == 0),
                            stop=False,
                        )
                transpose_half(q, t, 1, "vector" if t == TT - 1 else "scalar")
                for sub in range(NSUB):
                    ps = pss[sub]
                    for kc in range(KCH, KC):
                        nc.tensor.matmul(
                            ps[:, :P],
                            lhsT=w0[:, kc, sub * P : (sub + 1) * P],
                            rhs=qT[:, t, kc, :],
                            start=False,
                            stop=(kc == KC - 1),
                        )
                    dequant_store(ps[:, :P], sub, t * P, (t + 1) * P)
        wtiles.pop(0)

        # ---- Steady state: streamed weights-stationary GEMM, 512-col ----
        HT = TT // 2
        for wc in range(1, WC):
            wtile = wtiles.pop(wc) if wc in wtiles else load_wchunk(wc)
            for sub in range(NSUB):
                nt = wc * NSUB + sub
                if wc == WC - 1 and sub == NSUB - 1:
                    # last psum group split in column halves so the first
                    # half dequant+store overlaps the second half's matmuls
                    # (trims the kernel tail)
                    psA = ps_pool.tile([P, S_C], F32, name="ps")
                    psB = ps_pool.tile([P, S_C], F32, name="ps")
                    for kc in range(KC):
                        nc.tensor.matmul(
                            psA[:, : S_C // 2],
                            lhsT=wtile[:, kc, sub * P : (sub + 1) * P],
                            rhs=qT[:, :HT, kc, :],
                            start=(kc == 0),
                            stop=(kc == KC - 1),
                        )
                    dequant_store(psA[:, : S_C // 2], nt, 0, S_C // 2)
                    for kc in range(KC):
                        nc.tensor.matmul(
                            psB[:, : S_C // 2],
                            lhsT=wtile[:, kc, sub * P : (sub + 1) * P],
                            rhs=qT[:, HT:, kc, :],
                            start=(kc == 0),
                            stop=(kc == KC - 1),
                        )
                    dequant_store(psB[:, : S_C // 2], nt, S_C // 2, S_C)
                    continue
                ps = ps_pool.tile([P, S_C], F32, name="ps")
                for kc in range(KC):
                    nc.tensor.matmul(
                        ps,
                        lhsT=wtile[:, kc, sub * P : (sub + 1) * P],
                        rhs=qT[:, :, kc, :],
                        start=(kc == 0),
                        stop=(kc == KC - 1),
                    )
                dequant_store(ps, nt, 0, S_C)

    return nc


def pack_inputs(input_tensor, weight, weight_scale, S_C, K, N, NSUB=4):
    """Host-side prep: shard x, pack weights to bf16 SBUF-chunk layout."""
    KC = K // P
    NT = N // P
    WC = NT // NSUB
    x = np.ascontiguousarray(input_tensor.reshape(-1, K))  # [S, K]
    w_bf = weight.astype(ml_dtypes.bfloat16)  # [N, K], int8 values exact
    # pack[wc, p, kc, n] = w[wc*NSUB*P + n, kc*P + p]
    wt = np.ascontiguousarray(
        w_bf.reshape(WC, NSUB * P, KC, P).transpose(0, 3, 2, 1)
    )
    ws = np.ascontiguousarray(
        weight_scale.reshape(NT, P).T.astype(np.float32)
    )  # [P, NT]
    return x, wt, ws


@lru_cache(maxsize=2)
def _compiled_nc(S_C, K, N, NSUB, exact_divide):
    return build_nc(S_C, K, N, NSUB=NSUB, exact_divide=exact_divide)


def run(input_tensor, weight, weight_scale, n_cores=NCORES, trace=False,
        exact_divide=True, NSUB=4):
    Sfull, K = input_tensor.shape[-2], input_tensor.shape[-1]
    N = weight.shape[0]
    S_C = Sfull // n_cores
    x, wt, ws = pack_inputs(input_tensor, weight, weight_scale, S_C, K, N, NSUB)
    nc = _compiled_nc(S_C, K, N, NSUB, exact_divide)
    in_maps = [
        {"x": np.ascontiguousarray(x[c * S_C : (c + 1) * S_C]), "wt": wt, "ws": ws}
        for c in range(n_cores)
    ]
    res = run_bass_kernel_spmd(nc, in_maps, core_ids=list(range(n_cores)), trace=trace)
    out = np.empty((Sfull, N), np.float32)
    for c in range(n_cores):
        out[c * S_C : (c + 1) * S_C] = res.results[c]["outT"].T
    return out[None], res


def kernel(input_tensor, weight, weight_scale):
    out, _ = run(
        np.asarray(input_tensor), np.asarray(weight), np.asarray(weight_scale)
    )
    return out
